# revision 40
# baseline (speedup 1.0000x reference)
"""Brute-force-free kNN graph (N=65536, D=3, k=12) on 8 Trainium2 NeuronCores.

Host sorts points along a Morton curve over rank-quantized coordinates, so
spatial neighbours land close in sorted order. Each 128-row block then only
scores a B=4096-wide window of sorted columns (vs all 65536 brute force):

Device (per core, 8192 sorted rows = 64 blocks):
  - PE computes s[p, j] = 2*x_row[p].x_col[j] - ||x_col[j]||^2 via K=4
    fp32r matmuls (1 cy/row at free size 512); s = ||x_row||^2 - dist.
  - A 4-level pairwise tensor_max tree (DVE takes level 0 from PSUM, Pool
    the SBUF mid-levels) reduces each window to 256 group-maxes (group g =
    window cols {g + 256*m}, 16 cols per group).
  - Group maxes stream back to DRAM; no on-device top-k at all.
Host:
  - picks the top-32 groups per row (a true neighbour's group can be
    outranked only by the <=12 better points, so its group ranks <=13 --
    margin 19 absorbs fp32r score rounding), rescores all 512 named
    columns with XLA-CPU-exact fp32 arithmetic, takes the stable top-12.
  - a sound grid certificate (ball of the found 12th distance must be
    covered by window-resident Morton cells) flags rows whose neighbours
    may fall outside the window; those get an exact host fallback.
"""

import os
import sys

import numpy as np

for _p in ("/root/.axon_site/_ro/trn_rl_repo", "/opt/trn_rl_repo"):
    try:
        import concourse  # noqa: F401

        break
    except ImportError:
        if os.path.isdir(_p) and _p not in sys.path:
            sys.path.append(_p)

import concourse.bacc as bacc
import concourse.mybir as mybir
import concourse.tile as tile
from concourse.bass_utils import run_bass_kernel_spmd

import ml_dtypes

BF16NP = np.dtype(ml_dtypes.bfloat16)

F32 = mybir.dt.float32
BF16 = mybir.dt.bfloat16

K_OUT = 12
N_CORES = 8
B = 1536  # window width per 128-row block
G = 2  # columns per group (window reduced to B/G group maxes)
TOPG = 48  # groups rescored per row (host-side selection)
PAD = B // 2 - 64  # sentinel padding each side of the sorted array
SENT_XY = 1.0e4  # sentinel coordinate (pads never win: score ~ -3e8)
SENT_SQ = 3.0e8


def build_knn_nc(R, W):
    """R rows per core, W = R - 128 + B moving columns (padded coords)."""
    assert R % 128 == 0
    nblk = R // 128
    NG = B // G  # group maxes per block (256)

    nc = bacc.Bacc(None, target_bir_lowering=False, debug=False)
    xw_d = nc.dram_tensor("xw", [45, W], BF16, kind="ExternalInput")
    xr_d = nc.dram_tensor("xr", [45, R], BF16, kind="ExternalInput")
    gm_d = nc.dram_tensor("gm", [R, NG], BF16, kind="ExternalOutput")

    with tile.TileContext(nc) as tc:
        with (
            tc.tile_pool(name="const", bufs=1) as cpool,
            tc.tile_pool(name="t0p", bufs=3) as t0p,
            tc.tile_pool(name="t1p", bufs=3) as t1p,
            tc.tile_pool(name="gmp", bufs=4) as gmp,
            tc.tile_pool(name="psum", bufs=2, space="PSUM") as psum_pool,
        ):
            xw = cpool.tile([128, W], BF16, tag="xw")
            xr = cpool.tile([128, R], BF16, tag="xr")
            # chunked loads so early blocks start before the tail arrives
            NCH = 4
            for q in range(NCH):
                cw = (W + NCH - 1) // NCH
                s = q * cw
                nc.scalar.dma_start(
                    out=xw[0:45, s : min(s + cw, W)],
                    in_=xw_d[:, s : min(s + cw, W)],
                )
                cr = R // NCH
                nc.scalar.dma_start(
                    out=xr[0:45, q * cr : (q + 1) * cr],
                    in_=xr_d[:, q * cr : (q + 1) * cr],
                )

            for blk in range(nblk):
                lhsT = xr[0:45, blk * 128 : (blk + 1) * 128]
                base = blk * 128
                psa = psum_pool.tile([128, 768], F32, tag="psa")
                psb = psum_pool.tile([128, 768], F32, tag="psb")
                for ps, pso, off, wdt in (
                    (psa, 0, 0, 512),
                    (psa, 512, 512, 256),
                    (psb, 0, 768, 512),
                    (psb, 512, 1280, 256),
                ):
                    nc.tensor.matmul(
                        ps[:, pso : pso + wdt],
                        lhsT,
                        xw[0:45, base + off : base + off + wdt],
                        start=True,
                        stop=True,
                    )
                # ACT evacuates the left 768 to SBUF; DVE folds in the right
                # 768 in a single op. Group g = {g + 768*m, m<2}.
                t0 = t0p.tile([128, 768], F32, tag="t0")
                nc.scalar.activation(
                    out=t0[:, :], in_=psa[:, :],
                    func=mybir.ActivationFunctionType.Copy,
                )
                gm = gmp.tile([128, NG], BF16, tag="gm")
                nc.vector.tensor_max(gm[:, :], t0[:, :], psb[:, :])
                nc.sync.dma_start(
                    out=gm_d[blk * 128 : (blk + 1) * 128, :], in_=gm[:, :]
                )

    nc.compile()
    return nc


# ---------------------------------------------------------------- host side


def _morton3(q):
    def part1by2(v):
        v = v.astype(np.uint64)
        v = (v | (v << np.uint64(32))) & np.uint64(0x1F00000000FFFF)
        v = (v | (v << np.uint64(16))) & np.uint64(0x1F0000FF0000FF)
        v = (v | (v << np.uint64(8))) & np.uint64(0x100F00F00F00F00F)
        v = (v | (v << np.uint64(4))) & np.uint64(0x10C30C30C30C30C3)
        v = (v | (v << np.uint64(2))) & np.uint64(0x1249249249249249)
        return v

    return part1by2(q[:, 0]) | (part1by2(q[:, 1]) << np.uint64(1)) | (
        part1by2(q[:, 2]) << np.uint64(2)
    )


def _bf16x3(a):
    """Split fp32 (4, n) into three bf16 planes summing ~exactly to a."""
    a0 = a.astype(BF16NP)
    r1 = (a - a0.astype(np.float32)).astype(np.float32)
    a1 = r1.astype(BF16NP)
    a2 = (r1 - a1.astype(np.float32)).astype(BF16NP)
    return a0, a1, a2


def host_prep(x):
    """Sort rows by Morton code of per-dim ranks; build padded device inputs.

    Scores are computed on-device as a K=45 bf16 matmul producing exactly
    -d^2: stationary rows w = [2x | -1 | -xsq_row] and moving rows
    m = [x | xsq_col | 1] are each split into three bf16 planes
    (w0+w1+w2 ~= w exactly); all 9 cross products accumulate in fp32 PSUM,
    giving ~fp32-accurate -d^2 at bf16 PE throughput. Relative bf16
    rounding of -d^2 is tie-safe for host top-TOPG selection (measured
    worst neighbour-group rank 26).
    """
    N = x.shape[0]
    R = N // N_CORES
    W = R - 128 + B
    ranks = np.empty((N, 3), np.uint64)
    for d in range(3):
        ranks[np.argsort(x[:, d], kind="stable"), d] = np.arange(N, dtype=np.uint64)
    order = np.argsort(_morton3(ranks), kind="stable").astype(np.int64)
    xs = x[order]  # (N, 3) sorted
    xsqs = (
        (xs[:, 0] * xs[:, 0] + xs[:, 1] * xs[:, 1]) + xs[:, 2] * xs[:, 2]
    ).astype(np.float32)

    NP = N + 2 * PAD
    xp = np.full((5, NP), SENT_XY, np.float32)
    xp[0:3, PAD : PAD + N] = xs.T
    xp[3, :] = SENT_SQ
    xp[3, PAD : PAD + N] = xsqs
    xp[4, :] = 1.0
    m0, m1, m2 = _bf16x3(xp)
    # moving K-blocks (i, j) lexicographic: block t uses m_{t%3}
    xw_full = np.concatenate([m0, m1, m2, m0, m1, m2, m0, m1, m2], axis=0)

    in_maps = []
    for c in range(N_CORES):
        rows = slice(c * R, (c + 1) * R)
        w = np.concatenate(
            [
                2.0 * xs[rows].T,
                np.full((1, R), -1.0, np.float32),
                -xsqs[rows][None, :],
            ],
            axis=0,
        ).astype(np.float32)
        w0, w1, w2 = _bf16x3(w)
        # stationary block t uses w_{t//3}
        xr = np.concatenate([w0, w0, w0, w1, w1, w1, w2, w2, w2], axis=0)
        xw = np.ascontiguousarray(xw_full[:, c * R : c * R + W])
        in_maps.append({"xw": xw, "xr": np.ascontiguousarray(xr)})
    return in_maps, order, ranks


def _exact_rescore(x, xsq64, gid, rows_orig):
    """XLA-CPU-exact distances for candidate ids gid (M, C); returns packed
    (dist_bits, id) int64 keys (self/invalid get the max key)."""
    x0, x1, x2 = x[:, 0], x[:, 1], x[:, 2]
    r = rows_orig
    m = (x0[r, None].astype(np.float64) * x0[gid]).astype(np.float32)
    m = (x1[r, None].astype(np.float64) * x1[gid] + m).astype(np.float32)
    m = (x2[r, None].astype(np.float64) * x2[gid] + m).astype(np.float32)
    A = (xsq64[r][:, None] + xsq64[gid]).astype(np.float32)
    dist = (A.astype(np.float64) - 2.0 * m.astype(np.float64)).astype(np.float32)
    np.maximum(dist, 0.0, out=dist)
    np.add(dist, 0.0, out=dist)  # flush -0.0 for bit-monotone keys
    key = dist.view(np.uint32).astype(np.int64) * 131072 + gid
    key[gid == r[:, None]] = np.int64(1) << 62
    return key


def _topk_from_keys(key, k):
    sel = np.argpartition(key, k, axis=1)[:, :k]
    skey = np.take_along_axis(key, sel, axis=1)
    o = np.argsort(skey, axis=1)
    skey = np.take_along_axis(skey, o, axis=1)
    idx = (skey & 131071).astype(np.int32)
    dist = (skey >> 17).astype(np.uint32).view(np.float32).astype(np.float32)
    return dist, idx


def host_finish(x, gm_all, order, ranks, k):
    """Select top groups, rescore exactly, certify, fall back where needed."""
    import time
    from concurrent.futures import ThreadPoolExecutor

    _prof = os.environ.get("KNN_PROF")
    _t0 = time.time()

    def _tick(name):
        nonlocal _t0
        if _prof:
            t = time.time()
            print(f"    [host_finish] {name}: {t - _t0:.2f}s", flush=True)
            _t0 = t

    N = x.shape[0]
    # fp32 stepwise like XLA-CPU (each square and add rounded to fp32)
    xsq64 = (
        (x[:, 0] * x[:, 0] + x[:, 1] * x[:, 1]) + x[:, 2] * x[:, 2]
    ).astype(np.float32).astype(np.float64)

    # --- candidate ids per sorted row: TOPG groups of G columns
    NG = B // G
    # largest TOPG groups per row (kth from the high end avoids negating)
    sel = np.argpartition(gm_all, NG - TOPG, axis=1)[:, NG - TOPG :]
    srow = np.arange(N, dtype=np.int64)
    wbase = (srow // 128) * 128  # window start, padded coords
    # padded col = wbase + group + 256*m
    pcol = (
        wbase[:, None, None]
        + sel[:, :, None]
        + (np.arange(G, dtype=np.int64) * NG)[None, None, :]
    ).reshape(N, TOPG * G)
    spos = pcol - PAD  # sorted position
    valid = (spos >= 0) & (spos < N)

    gid = np.empty((N, TOPG * G), np.int32)
    rows_orig = order.astype(np.int32)  # sorted row -> original id
    np.copyto(gid, rows_orig[:, None])  # invalid -> self (masked by key rule)
    gid[valid] = order[spos[valid].astype(np.int64)].astype(np.int32)

    out_d = np.empty((N, k), np.float32)
    out_i = np.empty((N, k), np.int32)

    CB = 4096

    def _do(s):
        e = min(s + CB, N)
        key = _exact_rescore(x, xsq64, gid[s:e], rows_orig[s:e])
        d, i = _topk_from_keys(key, k)
        out_d[rows_orig[s:e]] = d
        out_i[rows_orig[s:e]] = i

    with ThreadPoolExecutor(max_workers=8) as ex:
        list(ex.map(_do, range(0, N, CB)))
    _tick("select+expand+rescore")

    # --- certificate (in original-id space): ball(x_i, rho_i) must be
    # covered by Morton cells entirely inside row i's window.
    # out_d holds SQUARED distances; the cert ball radius is its sqrt
    rho = np.sqrt(out_d[:, k - 1].astype(np.float64)) * (1 + 1e-6) + 1e-12
    LB = 5  # cert grid: 2^LB bins per dim
    SH = 16 - LB
    pos_of = np.empty(N, np.int64)  # original id -> sorted position
    pos_of[order] = srow
    wlo = (pos_of // 128) * 128 - PAD  # window range in sorted positions
    whi = wlo + B  # exclusive

    cid_pts = _morton3((ranks >> np.uint64(SH)).astype(np.uint64)).astype(np.int64)
    NCELL = 1 << (3 * LB)
    cmin = np.full(NCELL, np.iinfo(np.int64).max, np.int64)
    cmax = np.full(NCELL, -1, np.int64)
    np.minimum.at(cmin, cid_pts, pos_of)
    np.maximum.at(cmax, cid_pts, pos_of)

    lob = np.empty((N, 3), np.int64)
    hib = np.empty((N, 3), np.int64)
    for d in range(3):
        sv = np.sort(x[:, d].astype(np.float64))
        lo = np.searchsorted(sv, x[:, d].astype(np.float64) - rho, "left")
        hi = np.searchsorted(sv, x[:, d].astype(np.float64) + rho, "right") - 1
        lob[:, d] = lo >> SH
        hib[:, d] = np.minimum(hi, N - 1) >> SH

    nb = hib - lob + 1
    MAXB = 6
    cert_ok = np.all(nb <= MAXB, axis=1)
    q = np.empty((N, 3), np.uint64)
    for dx in range(MAXB):
        for dy in range(MAXB):
            for dz in range(MAXB):
                m = (
                    cert_ok
                    & (dx < nb[:, 0])
                    & (dy < nb[:, 1])
                    & (dz < nb[:, 2])
                )
                if not m.any():
                    continue
                q[m, 0] = (lob[m, 0] + dx).astype(np.uint64)
                q[m, 1] = (lob[m, 1] + dy).astype(np.uint64)
                q[m, 2] = (lob[m, 2] + dz).astype(np.uint64)
                cell = _morton3(q[m]).astype(np.int64)
                cm, cM = cmin[cell], cmax[cell]
                ok = (cm > cM) | ((cm >= wlo[m]) & (cM < whi[m]))
                mm = m.copy()
                mm[m] = ~ok
                cert_ok[mm] = False

    fb = np.where(~cert_ok)[0]
    _tick("cert")
    LAST_STATS["fallback_rows"] = int(fb.size)
    if fb.size:
        # exact fallback: fp32 approximate distances against all points,
        # then block-hierarchical selection (top-24 blocks of 128 cols --
        # only <=12 better points can outrank a true neighbour's block)
        # and exact rescore. Avoids 65536-wide argpartition entirely.
        xsq32 = xsq64.astype(np.float32)
        xT = np.ascontiguousarray(x.T)
        NB = N // 128
        ar128 = np.arange(128, dtype=np.int32)
        FCB = 512

        def _fb_do(s):
            e = min(s + FCB, fb.size)
            rows = fb[s:e]
            d2 = x[rows] @ xT
            d2 *= -2.0
            d2 += xsq32[rows][:, None]
            d2 += xsq32[None, :]
            d2[np.arange(rows.size), rows] = np.inf
            bm = d2.reshape(rows.size, NB, 128).min(axis=2)
            bsel = np.argpartition(bm, 24, axis=1)[:, :24].astype(np.int32)
            cand = (
                bsel[:, :, None] * 128 + ar128[None, None, :]
            ).reshape(rows.size, 24 * 128)
            key = _exact_rescore(x, xsq64, cand, rows.astype(np.int32))
            d, i = _topk_from_keys(key, k)
            out_d[rows] = d
            out_i[rows] = i

        for s in range(0, fb.size, FCB):
            _fb_do(s)  # serial: BLAS already multithreads the big sgemm
    _tick("fallback")
    return out_d, out_i


_NC_CACHE = {}
LAST_STATS = {}


def kernel(x, k, chunk_size):
    x = np.ascontiguousarray(np.asarray(x, dtype=np.float32))
    N = x.shape[0]
    R = N // N_CORES
    W = R - 128 + B
    key = (N, R)
    if key not in _NC_CACHE:
        _NC_CACHE[key] = build_knn_nc(R, W)
    nc = _NC_CACHE[key]
    in_maps, order, ranks = host_prep(x)
    res = run_bass_kernel_spmd(nc, in_maps, list(range(N_CORES)))
    gm_all = np.concatenate(
        [res.results[c]["gm"] for c in range(N_CORES)], axis=0
    ).astype(np.float32)
    return host_finish(x, gm_all, order, ranks, int(k))
